# revision 1
# baseline (speedup 1.0000x reference)
"""Trainium2 Bass kernel for the HardCL contrastive loss (nn_HardCL).

Math (reference, with BETA=1, ESTIMATOR="hard", TEMPERATURE=0.5, TAU_PLUS=0.1):
    out  = concat(out_1, out_2)                    # [2B, d], rows L2-normalized
    sim  = exp(out @ out.T / T)                    # [2B, 2B]
    row r masks cols {r%B, r%B+B} (self + positive pair)
    pos  = exp(dot(out_1_r, out_2_r) / T)
    With beta=1:  imp = neg, so
      reweight = sum(neg^2) / (sum(neg)/N),  N = 2B-2
      Ng   = max((-tau*N*pos + reweight)/(1-tau), N*e^{-1/T})
      loss = mean(-log(pos / (pos + Ng)))

Key identities used on device:
    masked row sum of s   = full row sum  - e^{1/T} - pos     (self dot == 1)
    masked row sum of s^2 = full row sum2 - e^{2/T} - pos^2

Sharding: core k owns rows [1024k, 1024k+1024) of the 2B=8192-row score
matrix. Every core receives the full gram operand out.T (bf16, K=128 on
partitions) plus its own 1024 columns as matmul weights; the all-gather is
done on the host by replication. Per-row log(pos/(pos+Ng)) comes back per
core; the final mean is a host-side gather.

Per-core pipeline (raw bass blocks, explicit semaphores):
    PE : bf16 matmuls [128,128] x [128,512] -> PSUM, 2048-wide chunks
    ACT: exp(2*dot) on each PSUM chunk with fused accum_out row-sum
    DVE: scalar_tensor_tensor(st*st) with accum_out -> row sums of s^2
    tiny fp32 per-row final math on ACT/DVE -> lossv [128, 8] per core
"""

import math

import ml_dtypes
import numpy as np

import concourse.bass as bass
import concourse.mybir as mybir
from concourse.bass_utils import run_bass_kernel_spmd

# ---- problem constants (hardcoded per contract) ----
B = 4096
D = 128
TWO_B = 2 * B                       # 8192
N_CORES = 8
ROWS_PER_CORE = TWO_B // N_CORES    # 1024
M_TILES = ROWS_PER_CORE // 128      # 8
CHUNK = 2048                        # ACT/DVE granularity = 4 PSUM banks
N_CHUNKS = TWO_B // CHUNK           # 4 chunks across a full row band
MM_N = 512                          # matmul moving free dim (1 PSUM bank)
N_STEPS = M_TILES * N_CHUNKS        # 32 pipelined chunk steps
ST_BUFS = 4                         # exp-tile buffers
ACT_SQ = {10, 22}                   # chunks whose square runs on ACT, not DVE

TAU = 0.1
TEMP = 0.5
NN = float(TWO_B - 2)               # 8190
E2 = math.exp(1.0 / TEMP)           # self-sim exp(dot/T), dot == 1
E4 = math.exp(2.0 / TEMP)
FLOOR = NN * math.exp(-1.0 / TEMP)
C_RW = NN / (1.0 - TAU)             # reweight scale N/(1-tau)
C_POS = -TAU * NN / (1.0 - TAU)

F32 = mybir.dt.float32
BF16 = mybir.dt.bfloat16
ALU = mybir.AluOpType
AF = mybir.ActivationFunctionType


def build_program() -> bass.Bass:
    nc = bass.Bass(trn_type="TRN2")

    gT = nc.declare_dram_parameter("gT", [128, TWO_B], BF16, isOutput=False)
    rT = nc.declare_dram_parameter("rT", [128, ROWS_PER_CORE], BF16, isOutput=False)
    o1p = nc.declare_dram_parameter("o1p", [128, ROWS_PER_CORE], BF16, isOutput=False)
    o2p = nc.declare_dram_parameter("o2p", [128, ROWS_PER_CORE], BF16, isOutput=False)
    lossv = nc.declare_dram_parameter("lossv", [128, M_TILES], F32, isOutput=True)

    from contextlib import ExitStack

    with ExitStack() as ctx:
        gT_s = ctx.enter_context(nc.sbuf_tensor([128, TWO_B], BF16))
        rT_s = ctx.enter_context(nc.sbuf_tensor([128, ROWS_PER_CORE], BF16))
        o1_s = ctx.enter_context(nc.sbuf_tensor([128, ROWS_PER_CORE], BF16))
        o2_s = ctx.enter_context(nc.sbuf_tensor([128, ROWS_PER_CORE], BF16))
        st_s = ctx.enter_context(nc.sbuf_tensor([128, ST_BUFS * CHUNK], BF16))
        sq_s = ctx.enter_context(nc.sbuf_tensor([128, CHUNK], BF16))
        sq2_s = ctx.enter_context(nc.sbuf_tensor([128, CHUNK], BF16))
        ssum = ctx.enter_context(nc.sbuf_tensor([128, N_STEPS], F32))
        s2sum = ctx.enter_context(nc.sbuf_tensor([128, N_STEPS], F32))
        pd_scr = ctx.enter_context(nc.sbuf_tensor([128, 128], BF16))
        fin = ctx.enter_context(nc.sbuf_tensor([128, 16 * M_TILES], F32))
        ps_s = ctx.enter_context(nc.psum_tensor([128, 2 * CHUNK], F32))
        sem_names = ["rT_sem", "o12_sem", "pe_sem", "act_sem", "v_sem",
                     "pd_sem", "pexp_sem", "rat_sem", "ln_sem"]
        (rT_sem, o12_sem, pe_sem, act_sem, v_sem, pd_sem, pexp_sem,
         rat_sem, ln_sem) = (
            ctx.enter_context(nc.semaphore(nm)) for nm in sem_names
        )
        g_sems = [
            ctx.enter_context(nc.semaphore(f"g{p}_sem")) for p in range(8)
        ]
        block = ctx.enter_context(nc.Block())
        PIECE = 1024  # gT DMA piece width

        # small [128, M_TILES] fp32 views into `fin`
        def f(i):
            return fin[:, i * M_TILES : (i + 1) * M_TILES]

        posd, pos, pos2, sneg, sneg2, rec, rw, ng = (f(i) for i in range(8))
        den, rden, ratio, ssum_t, s2sum_t, out_t = (f(i) for i in range(8, 14))

        st = [st_s[:, k * CHUNK : (k + 1) * CHUNK] for k in range(ST_BUFS)]
        ps = [ps_s[:, 0:CHUNK], ps_s[:, CHUNK : 2 * CHUNK]]

        def dma_piece(eng, p):
            eng.dma_start(
                gT_s[:, p * PIECE : (p + 1) * PIECE],
                gT[:, p * PIECE : (p + 1) * PIECE],
            ).then_inc(g_sems[p], 16)

        @block.sync
        def _(sync):
            sync.dma_start(rT_s[:, :], rT[:, :]).then_inc(rT_sem, 16)
            dma_piece(sync, 1)
            dma_piece(sync, 7)
            sync.wait_ge(ln_sem, 1)
            sync.dma_start(lossv[:, :], out_t).then_inc(rT_sem, 16)

        @block.gpsimd
        def _(gpsimd):
            # spread input DMAs over every engine's queue so they overlap
            gpsimd.dma_start(o1_s[:, :], o1p[:, :]).then_inc(o12_sem, 16)
            gpsimd.dma_start(o2_s[:, :], o2p[:, :]).then_inc(o12_sem, 16)
            dma_piece(gpsimd, 2)
            dma_piece(gpsimd, 3)
            dma_piece(gpsimd, 6)

        @block.tensor
        def _(tensor):
            tensor.wait_ge(rT_sem, 16)
            for i in range(N_STEPS):
                c, t = divmod(i, M_TILES)   # column-major: c outer, t inner
                if t == 0:  # gT pieces for chunk c must have arrived
                    tensor.wait_ge(g_sems[2 * c], 16)
                    tensor.wait_ge(g_sems[2 * c + 1], 16)
                if i >= 2:
                    tensor.wait_ge(act_sem, i - 1)   # PSUM buffer recycle
                mm = None
                for j in range(CHUNK // MM_N):
                    n0 = c * CHUNK + j * MM_N
                    mm = nc.tensor.matmul(
                        ps[i % 2][:, j * MM_N : (j + 1) * MM_N],
                        rT_s[:, t * 128 : (t + 1) * 128],
                        gT_s[:, n0 : n0 + MM_N],
                        start=True,
                        stop=True,
                    )
                mm.then_inc(pe_sem, 1)

        @block.scalar
        def _(scalar):
            dma_piece(scalar, 0)
            dma_piece(scalar, 4)
            dma_piece(scalar, 5)
            for i in range(N_STEPS):
                scalar.wait_ge(pe_sem, i + 1)
                if i >= ST_BUFS:
                    # st buffer recycle; +M_TILES: v_sem counts pair dots first
                    scalar.wait_ge(v_sem, M_TILES + i - (ST_BUFS - 1))
                nc.scalar.activation(
                    out=st[i % ST_BUFS],
                    in_=ps[i % 2][:, :],
                    func=AF.Exp,
                    scale=2.0,
                    accum_out=ssum[:, i : i + 1],
                ).then_inc(act_sem, 1)
                if i in ACT_SQ:
                    # balance: square+reduce this chunk on ACT instead of DVE
                    # (self-wait = explicit same-engine RAW edge on st)
                    scalar.wait_ge(act_sem, i + 1)
                    nc.scalar.activation(
                        out=sq2_s[:, :],
                        in_=st[i % ST_BUFS],
                        func=AF.Square,
                        accum_out=s2sum[:, i : i + 1],
                    )
                if i == 2:
                    # pos = exp(2 * pair_dot), early so the tail is short
                    scalar.wait_ge(pd_sem, 1)
                    nc.scalar.activation(
                        out=pos, in_=posd, func=AF.Exp, scale=2.0
                    ).then_inc(pexp_sem, 1)
            # final log
            scalar.wait_ge(rat_sem, 1)
            nc.scalar.activation(out=out_t, in_=ratio, func=AF.Ln).then_inc(ln_sem, 1)

        @block.vector
        def _(vector):
            # every DVE op chains v_sem so the race detector sees an explicit
            # same-engine ordering edge
            vcount = [0]

            def vchain(inst):
                inst.then_inc(v_sem, 1)
                vcount[0] += 1

            def vwait():
                if vcount[0]:
                    vector.wait_ge(v_sem, vcount[0])

            # pair dots first (inputs arrive early; DVE is idle anyway):
            # posd[p, t] = sum_d o1[r, d] * o2[r, d], r = t*128+p
            vector.wait_ge(o12_sem, 32)
            for t in range(M_TILES):
                vwait()
                vchain(nc.vector.scalar_tensor_tensor(
                    out=pd_scr[:, :],
                    in0=o1_s[:, t * 128 : (t + 1) * 128],
                    scalar=1.0,
                    in1=o2_s[:, t * 128 : (t + 1) * 128],
                    op0=ALU.mult,
                    op1=ALU.mult,
                    accum_out=posd[:, t : t + 1],
                ))
            nc.vector.engine_nop().then_inc(pd_sem, 1)
            for i in range(N_STEPS):
                vector.wait_ge(act_sem, i + 1)
                if i in ACT_SQ:
                    # ACT handles this chunk's square; keep v_sem counting
                    vchain(nc.vector.engine_nop())
                    continue
                vwait()  # same-engine WAW edge on sq_s
                vchain(nc.vector.scalar_tensor_tensor(
                    out=sq_s[:, :],
                    in0=st[i % ST_BUFS],
                    scalar=1.0,
                    in1=st[i % ST_BUFS],
                    op0=ALU.mult,
                    op1=ALU.mult,
                    accum_out=s2sum[:, i : i + 1],
                ))
            # per-band totals (ssum writes are covered by act_sem >= 32 above)
            vwait()
            vchain(nc.vector.tensor_reduce(
                out=ssum_t,
                in_=ssum.rearrange("p (c t) -> p t c", t=M_TILES),
                axis=mybir.AxisListType.X,
                op=ALU.add,
            ))
            vwait()
            vchain(nc.vector.tensor_reduce(
                out=s2sum_t,
                in_=s2sum.rearrange("p (c t) -> p t c", t=M_TILES),
                axis=mybir.AxisListType.X,
                op=ALU.add,
            ))
            vector.wait_ge(pexp_sem, 1)
            vwait()
            vchain(nc.vector.tensor_mul(pos2, pos, pos))
            # masked sums via analytic subtraction of self + pair terms
            vwait()
            vchain(nc.vector.scalar_tensor_tensor(
                out=sneg, in0=ssum_t, scalar=-E2, in1=pos,
                op0=ALU.add, op1=ALU.subtract,
            ))
            vwait()
            vchain(nc.vector.scalar_tensor_tensor(
                out=sneg2, in0=s2sum_t, scalar=-E4, in1=pos2,
                op0=ALU.add, op1=ALU.subtract,
            ))
            vwait()
            vchain(nc.vector.reciprocal(out=rec, in_=sneg))
            vwait()
            vchain(nc.vector.scalar_tensor_tensor(
                out=rw, in0=sneg2, scalar=C_RW, in1=rec,
                op0=ALU.mult, op1=ALU.mult,
            ))
            vwait()
            vchain(nc.vector.scalar_tensor_tensor(
                out=ng, in0=pos, scalar=C_POS, in1=rw,
                op0=ALU.mult, op1=ALU.add,
            ))
            vwait()
            vchain(nc.vector.tensor_scalar_max(ng, ng, FLOOR))
            vwait()
            vchain(nc.vector.tensor_add(den, pos, ng))
            vwait()
            vchain(nc.vector.reciprocal(out=rden, in_=den))
            vwait()
            vchain(nc.vector.tensor_mul(ratio, pos, rden))
            nc.vector.engine_nop().then_inc(rat_sem, 1)

    return nc


_NC_CACHE: dict = {}


def _get_nc() -> bass.Bass:
    if "nc" not in _NC_CACHE:
        _NC_CACHE["nc"] = build_program()
    return _NC_CACHE["nc"]


def make_in_maps(out_1: np.ndarray, out_2: np.ndarray) -> list[dict]:
    out = np.concatenate([out_1, out_2], axis=0)                # [8192, 128]
    gT = np.ascontiguousarray(out.T).astype(ml_dtypes.bfloat16)  # [128, 8192]
    in_maps = []
    for k in range(N_CORES):
        r0 = k * ROWS_PER_CORE
        rT_k = np.ascontiguousarray(gT[:, r0 : r0 + ROWS_PER_CORE])
        idx = np.arange(r0, r0 + ROWS_PER_CORE) % B
        o1blk = out_1[idx].astype(ml_dtypes.bfloat16)           # [1024, 128]
        o2blk = out_2[idx].astype(ml_dtypes.bfloat16)
        # pack: column t*128+d on partition p holds row (t*128+p), feature d
        o1p_k = np.ascontiguousarray(
            o1blk.reshape(M_TILES, 128, D).transpose(1, 0, 2).reshape(128, ROWS_PER_CORE)
        )
        o2p_k = np.ascontiguousarray(
            o2blk.reshape(M_TILES, 128, D).transpose(1, 0, 2).reshape(128, ROWS_PER_CORE)
        )
        in_maps.append({"gT": gT, "rT": rT_k, "o1p": o1p_k, "o2p": o2p_k})
    return in_maps


def run(out_1: np.ndarray, out_2: np.ndarray, trace: bool = False):
    """Run the SPMD kernel; returns (loss_scalar, BassKernelResults)."""
    in_maps = make_in_maps(out_1, out_2)
    nc = _get_nc()
    res = run_bass_kernel_spmd(
        nc, in_maps, core_ids=list(range(N_CORES)), trace=trace
    )
    logs = np.stack([res.results[k]["lossv"] for k in range(N_CORES)])
    loss = -np.mean(logs.astype(np.float64))
    return np.asarray(loss, dtype=np.float32), res


def kernel(out_1: np.ndarray, out_2: np.ndarray) -> np.ndarray:
    loss, _ = run(np.asarray(out_1), np.asarray(out_2), trace=False)
    return loss

